# revision 1
# baseline (speedup 1.0000x reference)
"""DeepSeek MoE gate routing kernel for Trainium2 (Bass/Tile), 8-core SPMD.

Problem: hidden_states [4, 4096, 4096] f32, gate weight [256, 4096] f32.
  logits = x @ W^T          (T=16384 tokens, E=256 experts, h=4096)
  scores = softmax(logits)
  topk_w, topk_i = top_k(scores, 8); topk_w = topk_w / sum(topk_w) * 2.5

Sharding: tokens split across 8 cores (2048 each); W replicated.

Per-core pipeline (16 tiles of 128 tokens):
  - DMA x tile [128, 4096] in natural layout (contiguous, full BW)
  - PE-transpose x chunks [128t,128k] -> [128k,128t] (batches of 4 into one
    PSUM bank), copy PSUM->SBUF (alternating DVE/ACT)
  - matmul accumulate logits [128, 256] over 32 k-chunks
    (stationary = x^T chunk, moving = W^T chunk [128, 256])
  - W^T built on-chip once via 64 PE transposes of W
  - top-8: nc.vector.max (InstMax, descending top-8) + max_index
  - weights: exp(top8 - max) on ACT, sum/reciprocal/scale on DVE
"""

import numpy as np

import concourse.bass as bass
import concourse.mybir as mybir
from concourse import bacc
from concourse.bass_utils import run_bass_kernel_spmd
from concourse.masks import make_identity
from concourse.tile import TileContext

N_CORES = 8
H = 4096            # hidden size
E = 256             # n experts
TOPK = 8
T_FULL = 4 * 4096   # 16384 tokens
T_CORE = T_FULL // N_CORES  # 2048
P = 128             # partitions
N_TILES = T_CORE // P       # 16
KCH = H // P                # 32 contraction chunks
SCALE = 2.5         # routed_scaling_factor

F32 = mybir.dt.float32

def build_bass(mm_dt=mybir.dt.float32):
    nc = bacc.Bacc(trn_type="TRN2")
    x = nc.dram_tensor("x", [T_CORE, H], F32, kind="ExternalInput")
    w = nc.dram_tensor("w", [E, H], F32, kind="ExternalInput")
    oid = nc.dram_tensor("oid", [T_CORE, TOPK], mybir.dt.int32, kind="ExternalOutput")
    owt = nc.dram_tensor("owt", [T_CORE, TOPK], F32, kind="ExternalOutput")

    with TileContext(nc) as tc:
        with (
            tc.tile_pool(name="const", bufs=1) as const_pool,
            tc.tile_pool(name="wnat", bufs=1) as wnat_pool,
            tc.tile_pool(name="wt", bufs=1) as wt_pool,
            tc.tile_pool(name="xin", bufs=4) as x_pool,
            tc.tile_pool(name="xt", bufs=10) as xt_pool,
            tc.tile_pool(name="pt", bufs=4, space="PSUM") as pt_pool,
            tc.tile_pool(name="pl", bufs=3, space="PSUM") as pl_pool,
            tc.tile_pool(name="small", bufs=2) as small_pool,
        ):
            ident = const_pool.tile([P, P], F32, tag="ident")
            make_identity(nc, ident)

            # x tile 0 DMA first so tile-0 transposes can start ASAP;
            # split each big load across both HWDGE rings (SP + ACT).
            xin0 = x_pool.tile([P, H], F32, tag="xin")
            # first 1024 columns as their own transfer so transpose batches
            # 0-1 can start before the rest of the tile lands
            nc.sync.dma_start(out=xin0[:, : H // 4], in_=x[0:P, : H // 4])
            nc.sync.dma_start(out=xin0[:, H // 4: H // 2], in_=x[0:P, H // 4: H // 2])
            nc.scalar.dma_start(out=xin0[:, H // 2:], in_=x[0:P, H // 2:])

            w0 = wnat_pool.tile([P, H], F32, tag="w0")
            w1 = wnat_pool.tile([P, H], F32, tag="w1")
            nc.sync.dma_start(out=w0[:, : H // 2], in_=w[0:P, : H // 2])
            nc.scalar.dma_start(out=w0[:, H // 2:], in_=w[0:P, H // 2:])
            nc.sync.dma_start(out=w1[:, : H // 2], in_=w[P:E, : H // 2])
            nc.scalar.dma_start(out=w1[:, H // 2:], in_=w[P:E, H // 2:])
            w_nat = (w0, w1)

            def transpose_batch(src, b, copy_on_vector):
                """PE-transpose chunks 4b..4b+3 of src into one PSUM bank,
                copy to a fresh SBUF xT tile [128, 512]; returns the tile."""
                pt = pt_pool.tile([P, 512], F32, tag="pt")
                for i in range(4):
                    c = 4 * b + i
                    nc.tensor.matmul(
                        pt[:, i * P:(i + 1) * P],
                        lhsT=src[:, c * P:(c + 1) * P],
                        rhs=ident,
                        is_transpose=True,
                        start=(i == 0),
                        stop=(i == 3),
                    )
                xT = xt_pool.tile([P, 512], mm_dt, tag="xt", name=f"xT_{b}")
                if copy_on_vector:
                    nc.vector.tensor_copy(xT, pt)
                else:
                    nc.scalar.copy(xT, pt)
                return xT

            # tile-0 x transposes first: they only need x0 (2 MiB) while the
            # W^T build below waits on the 4 MiB weight load.
            t0_xT = [transpose_batch(xin0, b, b % 2 == 0) for b in range(KCH // 4)]

            # ---- one-time: build W^T [h, e] in SBUF as 32 chunks [128, 256].
            # e-major order: all expert-half-0 batches first (need only w0,
            # which lands before w1), then half-1.
            wT = wt_pool.tile([P, KCH * E], mm_dt, tag="wt")
            wT_r = wT.rearrange("p (c eh) -> p c eh", eh=E)
            for e in range(2):
                for b in range(KCH // 4):  # 8 batches of 4 chunks each
                    pt = pt_pool.tile([P, 512], F32, tag="pt")
                    for i in range(4):
                        c = 4 * b + i
                        nc.tensor.matmul(
                            pt[:, i * P:(i + 1) * P],
                            lhsT=w_nat[e][:, c * P:(c + 1) * P],
                            rhs=ident,
                            is_transpose=True,
                            start=(i == 0),
                            stop=(i == 3),
                        )
                    dst = wT_r[:, 4 * b:4 * b + 4, e * P:(e + 1) * P]
                    if b % 2 == 0:
                        nc.vector.tensor_copy(dst, pt.rearrange("p (c q) -> p c q", q=P))
                    else:
                        nc.scalar.copy(dst, pt.rearrange("p (c q) -> p c q", q=P))

            # ---- main loop over 16 token tiles ----
            for t in range(N_TILES):
                if t == 0:
                    xin = xin0
                else:
                    xin = x_pool.tile([P, H], F32, tag="xin")
                    nc.sync.dma_start(
                        out=xin[:, : H // 2], in_=x[t * P:(t + 1) * P, : H // 2]
                    )
                    nc.scalar.dma_start(
                        out=xin[:, H // 2:], in_=x[t * P:(t + 1) * P, H // 2:]
                    )
                logits_ps = pl_pool.tile([P, E], F32, tag="logits")
                for b in range(KCH // 4):  # 8 batches of 4 chunks
                    if t == 0:
                        xT = t0_xT[b]
                    else:
                        xT = transpose_batch(xin, b, b % 2 == 0)
                    for i in range(4):
                        c = 4 * b + i
                        nc.tensor.matmul(
                            logits_ps,
                            lhsT=xT[:, i * P:(i + 1) * P],
                            rhs=wT[:, c * E:(c + 1) * E],
                            start=(c == 0),
                            stop=(c == KCH - 1),
                        )

                # ---- top-8 + softmax-normalized weights (straight off PSUM) ----
                mx = small_pool.tile([P, TOPK], F32, tag="mx")
                nc.vector.max(out=mx, in_=logits_ps)
                idx = small_pool.tile([P, TOPK], mybir.dt.uint32, tag="idx")
                nc.vector.max_index(out=idx, in_max=mx, in_values=logits_ps)
                negm = small_pool.tile([P, 1], F32, tag="negm")
                nc.vector.tensor_scalar_mul(negm, mx[:, 0:1], -1.0)
                e8 = small_pool.tile([P, TOPK], F32, tag="e8")
                nc.scalar.activation(
                    e8, mx, mybir.ActivationFunctionType.Exp, bias=negm, scale=1.0
                )
                s8 = small_pool.tile([P, 1], F32, tag="s8")
                nc.vector.reduce_sum(s8, e8, axis=mybir.AxisListType.X)
                rcp = small_pool.tile([P, 1], F32, tag="rcp")
                nc.vector.reciprocal(rcp, s8)
                wt8 = small_pool.tile([P, TOPK], F32, tag="wt8")
                nc.vector.tensor_scalar(
                    wt8, e8, scalar1=rcp, scalar2=SCALE,
                    op0=mybir.AluOpType.mult, op1=mybir.AluOpType.mult,
                )
                nc.scalar.dma_start(
                    out=oid[t * P:(t + 1) * P, :], in_=idx.bitcast(mybir.dt.int32)
                )
                nc.scalar.dma_start(out=owt[t * P:(t + 1) * P, :], in_=wt8)
    nc.compile()
    return nc


_NC_CACHE = {}


def _get_nc(mm_dt=mybir.dt.float32):
    key = str(mm_dt)
    if key not in _NC_CACHE:
        _NC_CACHE[key] = build_bass(mm_dt)
    return _NC_CACHE[key]


def _ensure_ntff_hook():
    """This image's antenv lacks axon_hooks; shim it with the boot's own
    ctypes NTFF hook so trace=True works (only used by our test harness)."""
    import sys
    import types
    try:
        import antenv.axon_hooks  # noqa: F401
        return
    except ImportError:
        pass
    try:
        from trn_agent_boot.trn_boot import _ntff_profile_via_ctypes
        hook = _ntff_profile_via_ctypes("/opt/axon/libaxon_pjrt.so")
    except Exception:
        hook = None
    mod = types.ModuleType("antenv.axon_hooks")
    mod.get_axon_ntff_profile_hook = lambda: hook
    mod.set_axon_ntff_profile_hook = lambda h: None
    sys.modules["antenv.axon_hooks"] = mod
    import antenv
    antenv.axon_hooks = mod


def run(hidden_states, weight, mm_dt=mybir.dt.float32, trace=False):
    """Run on 8 NeuronCores; returns (topk_idx int32 [T,8], topk_w f32 [T,8], results)."""
    if trace:
        _ensure_ntff_hook()
    x = np.ascontiguousarray(
        np.asarray(hidden_states, dtype=np.float32).reshape(-1, H)
    )
    w = np.ascontiguousarray(np.asarray(weight, dtype=np.float32))
    assert x.shape == (T_FULL, H) and w.shape == (E, H)
    nc = _get_nc(mm_dt)
    in_maps = [
        {"x": np.ascontiguousarray(x[i * T_CORE:(i + 1) * T_CORE]), "w": w}
        for i in range(N_CORES)
    ]
    res = run_bass_kernel_spmd(
        nc, in_maps, core_ids=list(range(N_CORES)), trace=trace
    )
    idx = np.concatenate([r["oid"] for r in res.results], axis=0).astype(np.int32)
    wts = np.concatenate([r["owt"] for r in res.results], axis=0).astype(np.float32)
    return idx, wts, res


def kernel(hidden_states, weight):
    idx, wts, _ = run(hidden_states, weight)
    return idx, wts

